# revision 3
# baseline (speedup 1.0000x reference)
"""Trainium2 Bass kernel (fused fp8 DoubleRow variant).

Math: out = per-segment log-softmax heads over a 3-layer Swish MLP.  The
first two LINEAR maps fold on the host:
    z1 = W1 @ (W_seed @ xs.T + W_node @ xn.T)
       = (W1 @ W_seed) @ xs.T + (W1 @ W_node) @ xn.T = A1 @ xs.T + B1 @ xn.T
so the device runs TWO matmul layers, not three: layer01 (K=256: xs/xn
DoubleRow planes, weights A1|B1) and layer2 (K=1024 over h1).

Perf model (HW-measured):
  - fp8e4 DoubleRow MM [K=256 x N=512]: ~213-245 ns in-stream.
    Per tile: 8 (layer01) + 32 (layer2) + 4 (node-score M=1) = 44 MMs.
  - engine drains cost ~600-700 ns per [128,512] chunk AND steal PE time
    roughly proportional to bytes read -> drains read PSUM as bf16-hi
    (bitcast + stride-2, i.e. bf16 truncation; exact silu input err <0.4%).
  - weights prescaled x16 on host (fp8 subnormal avoidance); ACT silu
    applies scale=1/16.  b1/b2 are zero in this problem: no-bias 2-chunk
    silu ops; a `biased` fallback keeps general correctness.
  - ACT is the co-bottleneck (~16 chunk-silus/tile); DVE does the segment
    mean-pool reduces (fp8 SBUF reads) + ns staging; Pool/GpSimd does
    weight DMAs only; node scores run on the PE as M=1 DR MMs.
  - a segment (4 tiles) is issued as one block so layer01 MMs of later
    tiles cover the silu-drain latency of earlier ones.

Numerics (numpy model incl. x16 prescale + bf16 truncation + fp8 rounding):
max rel err 7.9e-4 vs f32 reference (gate 2e-2).
"""

import numpy as np

import concourse.bass as bass
import concourse.mybir as mybir
import concourse.tile as tile
from concourse import bacc
from concourse.bass import ts
from concourse.bass_utils import run_bass_kernel_spmd

B, L, H, D = 64, 2048, 1024, 128
NCORES = 8
B_LOC = B // NCORES          # segments per core
N_LOC = B_LOC * L            # tokens per core
T = 512                      # tokens per tile
NT = N_LOC // T              # tiles per core
U = L // T                   # tiles per segment
HC = H // 128                # hidden chunks
KP = HC // 2                 # k-pair chunks (256-wide DoubleRow contractions)
PRE = 16.0                   # host weight prescale (silu scale = 1/PRE)

F32 = mybir.dt.float32
BF16 = mybir.dt.bfloat16
FP8 = mybir.dt.float8e4
AF = mybir.ActivationFunctionType
AX = mybir.AxisListType
DR = mybir.MatmulPerfMode.DoubleRow

_NC_CACHE = {}


def _hi_bf16(pm):
    """View a [128, c, T] f32 PSUM tile as its bf16 high halves (truncation)."""
    return pm.bitcast(BF16).rearrange("p c (t two) -> p c t two",
                                      two=2)[:, :, :, 1]


def build_nc(reps=1, **vkw):
    key = (reps, tuple(sorted(vkw.items())))
    if key in _NC_CACHE:
        return _NC_CACHE[key]
    pmm_bufs = vkw.get("pmm_bufs", 3)   # [128, 2, T] f32: 2 banks each
    pns_bufs = vkw.get("pns_bufs", 2)
    raw_bufs = vkw.get("raw_bufs", 8)
    h1_bufs = vkw.get("h1_bufs", 5)
    h2_bufs = vkw.get("h2_bufs", 5)
    nsx_bufs = vkw.get("nsx_bufs", 3)
    biased = vkw.get("biased", False)
    no_pool = vkw.get("no_pool", False)
    no_ns = vkw.get("no_ns", False)
    no_heads = vkw.get("no_heads", False)
    defer = vkw.get("defer", True)
    if defer:
        h2_bufs = vkw.get("h2_bufs", 9)
    wide = vkw.get("wide", False)   # [128,4,T] PSUM tiles, silu4 drains
    if wide:
        pmm_bufs = vkw.get("pmm_bufs", 2)
        pns_bufs = 0

    nc = bacc.Bacc("TRN2", target_bir_lowering=False, debug=False,
                   num_devices=NCORES)

    x_d = nc.dram_tensor("xb", [128, NT, 2, T], FP8, kind="ExternalInput").ap()
    wd_d = nc.dram_tensor("wd", [128, 2, H], FP8, kind="ExternalInput").ap()
    w2_d = nc.dram_tensor("w2p", [128, KP, 2, H], FP8,
                          kind="ExternalInput").ap()
    b1_d = nc.dram_tensor("b1r", [128, HC], F32, kind="ExternalInput").ap()
    b2_d = nc.dram_tensor("b2r", [128, HC], F32, kind="ExternalInput").ap()
    wns8_d = nc.dram_tensor("wns8", [128, KP, 2, 16], FP8,
                            kind="ExternalInput").ap()
    wsp_d = nc.dram_tensor("wstr", [128, HC, 2], F32,
                           kind="ExternalInput").ap()
    out_d = nc.dram_tensor("out", [B_LOC, L + 1], F32,
                           kind="ExternalOutput").ap()

    with tile.TileContext(nc) as tc:
        with (
            tc.tile_pool(name="const", bufs=1) as cpool,
            tc.tile_pool(name="raw", bufs=raw_bufs) as raw,
            tc.tile_pool(name="h1", bufs=h1_bufs) as h1p,
            tc.tile_pool(name="h2", bufs=h2_bufs) as h2p,
            tc.tile_pool(name="acc", bufs=2) as accp,
            tc.tile_pool(name="nsx", bufs=nsx_bufs) as nsxp,
            tc.tile_pool(name="nsb", bufs=1) as nsp,
            tc.tile_pool(name="head", bufs=1) as headp,
            tc.tile_pool(name="pmm", bufs=pmm_bufs, space="PSUM") as pmm,
            tc.tile_pool(name="pns", bufs=max(pns_bufs, 1),
                         space="PSUM") as pnsp,
        ):
            if wide:
                pnsp = None
            wd = cpool.tile([128, 2, H], FP8)
            nc.gpsimd.dma_start(out=wd, in_=wd_d)
            w2 = cpool.tile([128, KP, 2, H], FP8)
            nc.gpsimd.dma_start(out=w2, in_=w2_d)
            b1 = cpool.tile([128, HC], F32)
            nc.gpsimd.dma_start(out=b1, in_=b1_d)
            b2 = cpool.tile([128, HC], F32)
            nc.gpsimd.dma_start(out=b2, in_=b2_d)
            wns8 = cpool.tile([128, KP, 2, 16], FP8)
            nc.gpsimd.dma_start(out=wns8, in_=wns8_d)
            wsp = cpool.tile([128, HC, 2], F32)
            nc.gpsimd.dma_start(out=wsp, in_=wsp_d)

            def load_x(t):
                xt = raw.tile([128, 2, T], FP8, tag="xt")
                nc.sync.dma_start(out=xt, in_=x_d[:, t, :, :])
                return xt

            def drain_silu(hf, g, pm, b, nch):
                # silu over nch chunks; PSUM read as truncated bf16
                if biased:
                    for c in range(nch):
                        m = nch * g + c
                        nc.scalar.activation(hf[:, m, :],
                                             _hi_bf16(pm)[:, c, :],
                                             AF.Silu, bias=b[:, m:m + 1],
                                             scale=1.0 / PRE)
                else:
                    nc.scalar.activation(hf[:, nch * g:nch * g + nch, :],
                                         _hi_bf16(pm), AF.Silu,
                                         scale=1.0 / PRE)

            def layer01(xt):
                # h1.T = silu((A1|B1) @ (xs|xn).T / PRE): one K=256 DR MM
                # per m-chunk, two chunks per 2-bank PSUM tile
                h1 = h1p.tile([128, KP, 2, T], FP8, tag="h1")
                h1f = h1.rearrange("p k two t -> p (k two) t")
                nch = 4 if wide else 2
                for g in range(HC // nch):
                    pm = pmm.tile([128, nch, T], F32, tag="mm")
                    for c in range(nch):
                        m = nch * g + c
                        nc.tensor.matmul(pm[:, c, :], wd[:, :, ts(m, 128)],
                                         xt, start=True, stop=True,
                                         perf_mode=DR)
                    drain_silu(h1f, g, pm, b1, nch)
                return h1

            def layer2(h1):
                h2 = h2p.tile([128, KP, 2, T], FP8, tag="h2")
                h2f = h2.rearrange("p k two t -> p (k two) t")
                nch = 4 if wide else 2
                for g in range(HC // nch):
                    pm = pmm.tile([128, nch, T], F32, tag="mm")
                    for c in range(nch):
                        m = nch * g + c
                        for k in range(KP):
                            nc.tensor.matmul(pm[:, c, :],
                                             w2[:, k, :, ts(m, 128)],
                                             h1[:, k, :, :],
                                             start=(k == 0),
                                             stop=(k == KP - 1),
                                             perf_mode=DR)
                    drain_silu(h2f, g, pm, b2, nch)
                return h2

            def pool_contrib(h2, stop_acc, u):
                # segment mean-pool: sum h2 over tokens (DVE, fp8 SBUF reads)
                h2f = h2.rearrange("p k two t -> p (k two) t")
                for m in range(HC):
                    nc.vector.reduce_sum(stop_acc[:, m, u:u + 1],
                                         h2f[:, m, :], axis=AX.X)

            def ns_chain(h2):
                # node scores on the PE: ns = w_ns . h2 via KP M=1 DR MMs
                if wide:
                    pnst = pmm.tile([128, 4, T], F32, tag="mm")
                    pns = pnst[0:1, 0, :]
                else:
                    pns = pnsp.tile([1, T], F32, tag="pns")
                for k in range(KP):
                    nc.tensor.matmul(pns, wns8[:, k, :, 0:1], h2[:, k, :, :],
                                     start=(k == 0), stop=(k == KP - 1),
                                     perf_mode=DR)
                return pns

            def ns_finish(pns, ns_all, s, u):
                ns_stage = nsxp.tile([1, T], F32, tag="ns_stage")
                nc.vector.tensor_copy(ns_stage, pns)
                nc.sync.dma_start(out=ns_all[s:s + 1, ts(u, T)], in_=ns_stage)

            def main_body():
                ns_all = nsp.tile([B_LOC, L], F32, tag="ns_all")
                stopT = nsp.tile([128, HC, B_LOC], F32, tag="stopT")

                def tail_work(h2s, stop_acc, s):
                    # consumers of segment s's h2 tiles: issued one segment
                    # late so the PE never waits on the silu drain tail
                    if no_pool:
                        nc.vector.memset(stop_acc, 0.0)
                    else:
                        for u in range(U):
                            pool_contrib(h2s[u], stop_acc, u)
                    if not no_ns:
                        pnss = [ns_chain(h2s[u]) for u in range(U)]
                        for u in range(U):
                            ns_finish(pnss[u], ns_all, s, u)
                    elif s == 0:
                        nc.vector.memset(ns_all, 0.0)
                    nc.vector.reduce_sum(stopT[:, :, s:s + 1], stop_acc,
                                         axis=AX.X)

                pend = [load_x(u) for u in range(U)]
                prev = None
                for s in range(B_LOC):
                    stop_acc = accp.tile([128, HC, U], F32, tag="stop_acc")
                    xts = pend
                    if s + 1 < B_LOC:
                        pend = [load_x((s + 1) * U + u) for u in range(U)]
                    h1s = [layer01(xts[u]) for u in range(U)]
                    if prev is not None:
                        tail_work(*prev)
                    h2s = [layer2(h1s[u]) for u in range(U)]
                    if defer:
                        prev = (h2s, stop_acc, s)
                    else:
                        tail_work(h2s, stop_acc, s)
                if prev is not None:
                    tail_work(*prev)

                if no_heads:
                    outsb0 = headp.tile([B_LOC, L + 1], F32, tag="outsb")
                    nc.vector.tensor_copy(outsb0[:, 0:L], ns_all)
                    nc.sync.dma_start(out=out_d, in_=outsb0)
                    return
                # ---- heads (batched over the 8 local segments) ----
                if wide:
                    pstt = pmm.tile([128, 4, T], F32, tag="mm")
                    pst = pstt[0:B_LOC, 0, 0:2]
                else:
                    pst = pnsp.tile([B_LOC, 2], F32, tag="pns")
                for k in range(HC):
                    nc.tensor.matmul(pst, stopT[:, k, :], wsp[:, k, :],
                                     start=(k == 0), stop=(k == HC - 1))
                st = headp.tile([B_LOC, 2], F32, tag="st")
                nc.scalar.mul(st, pst, 1.0 / L)
                negm = headp.tile([B_LOC, 1], F32, tag="negm")
                nc.vector.reduce_max(negm, st, axis=AX.X, negate=True)
                est = headp.tile([B_LOC, 2], F32, tag="est")
                sst = headp.tile([B_LOC, 1], F32, tag="sst")
                nc.scalar.activation(est, st, AF.Exp, bias=negm, scale=1.0,
                                     accum_out=sst)
                lst = headp.tile([B_LOC, 1], F32, tag="lst")
                nc.scalar.activation(lst, sst, AF.Ln)
                stop0 = headp.tile([B_LOC, 1], F32, tag="stop0")
                nc.vector.tensor_add(stop0, st[:, 0:1], negm)
                stop0b = headp.tile([B_LOC, 1], F32, tag="stop0b")
                nc.vector.tensor_sub(stop0b, stop0, lst)
                stop1 = headp.tile([B_LOC, 1], F32, tag="stop1")
                nc.vector.tensor_add(stop1, st[:, 1:2], negm)
                stop1b = headp.tile([B_LOC, 1], F32, tag="stop1b")
                nc.vector.tensor_sub(stop1b, stop1, lst)

                negnm = headp.tile([B_LOC, 1], F32, tag="negnm")
                nc.vector.reduce_max(negnm, ns_all, axis=AX.X, negate=True)
                esc = headp.tile([B_LOC, L], F32, tag="esc")
                nsum = headp.tile([B_LOC, 1], F32, tag="nsum")
                nc.scalar.activation(esc, ns_all, AF.Exp, bias=negnm,
                                     scale=1.0, accum_out=nsum)
                nls = headp.tile([B_LOC, 1], F32, tag="nls")
                nc.scalar.activation(nls, nsum, AF.Ln)
                fb = headp.tile([B_LOC, 1], F32, tag="fb")
                nc.vector.tensor_add(fb, stop0b, negnm)
                fb2 = headp.tile([B_LOC, 1], F32, tag="fb2")
                nc.vector.tensor_sub(fb2, fb, nls)

                outsb = headp.tile([B_LOC, L + 1], F32, tag="outsb")
                nc.scalar.activation(outsb[:, 0:L], ns_all, AF.Identity,
                                     bias=fb2, scale=1.0)
                nc.vector.tensor_copy(outsb[:, L:L + 1], stop1b)
                nc.sync.dma_start(out=out_d, in_=outsb)

            if reps == 1:
                main_body()
            else:
                with tc.For_i(0, reps, 1) as _i:
                    main_body()

    nc.compile()
    _NC_CACHE[key] = nc
    return nc


def _pad16(a):
    out = np.zeros(a.shape + (16,), dtype=a.dtype)
    out[..., 0] = a
    return out


def _prep_in_maps(x_seeds, x_nodes, W_seed, W_node, W1, b1, W2, b2, w_ns,
                  W_stop):
    import ml_dtypes
    E4 = ml_dtypes.float8_e4m3
    f32 = lambda a: np.asarray(a, dtype=np.float32)
    f32c = lambda a: np.ascontiguousarray(f32(a))

    # fold layers 0+1: A1 = W1 @ W_seed, B1 = W1 @ W_node  (x PRE prescale)
    A1 = (f32(W1) @ f32(W_seed)) * PRE        # [H, D]
    B1 = (f32(W1) @ f32(W_node)) * PRE
    wd = np.stack([A1.T, B1.T], axis=1)       # [D, 2, H]
    w2p = ((f32(W2) * PRE).T.reshape(KP, 2, 128, H).transpose(2, 0, 1, 3))
    shared = {
        "wd": np.ascontiguousarray(wd.astype(E4)),
        "w2p": np.ascontiguousarray(w2p.astype(E4)),
        "b1r": f32c(f32(b1).reshape(HC, 128).T),
        "b2r": f32c(f32(b2).reshape(HC, 128).T),
        "wns8": _pad16(f32(w_ns).reshape(KP, 2, 128).transpose(2, 0, 1)
                       .astype(E4)),
        "wstr": f32c(f32(W_stop).reshape(2, HC, 128).transpose(2, 1, 0)),
    }
    xs_q = f32(x_seeds).astype(E4)
    xn_q = f32(x_nodes).astype(E4)
    in_maps = []
    for cidx in range(NCORES):
        sl = slice(cidx * N_LOC, (cidx + 1) * N_LOC)
        a = xs_q[sl].reshape(NT, T, D).transpose(2, 0, 1)   # [128, NT, T]
        b = xn_q[sl].reshape(NT, T, D).transpose(2, 0, 1)
        xb = np.ascontiguousarray(np.stack([a, b], axis=2))  # [128, NT, 2, T]
        m = {"xb": xb}
        m.update(shared)
        in_maps.append(m)
    biased = bool(np.any(np.asarray(b1)) or np.any(np.asarray(b2)))
    return in_maps, biased


def run_on_hw(in_maps, reps=1, biased=False):
    nc = build_nc(reps, **({"biased": True} if biased else {}))
    res = run_bass_kernel_spmd(nc, in_maps, core_ids=list(range(NCORES)))
    return res


def kernel(x_seeds, x_nodes, W_seed, W_node, W1, b1, W2, b2, w_ns, W_stop,
           indptr=None, **_unused):
    in_maps, biased = _prep_in_maps(x_seeds, x_nodes, W_seed, W_node, W1, b1,
                                    W2, b2, w_ns, W_stop)
    res = run_on_hw(in_maps, biased=biased)
    out = np.concatenate([res.results[c]["out"] for c in range(NCORES)],
                         axis=0)
    return out.astype(np.float32)


# revision 4
# speedup vs baseline: 1.0573x; 1.0573x over previous
"""Trainium2 Bass kernel (fused fp8 DoubleRow variant).

Math: out = per-segment log-softmax heads over a 3-layer Swish MLP.  The
first two LINEAR maps fold on the host:
    z1 = W1 @ (W_seed @ xs.T + W_node @ xn.T)
       = (W1 @ W_seed) @ xs.T + (W1 @ W_node) @ xn.T = A1 @ xs.T + B1 @ xn.T
so the device runs TWO matmul layers, not three: layer01 (K=256: xs/xn
DoubleRow planes, weights A1|B1) and layer2 (K=1024 over h1).

Perf model (HW-measured):
  - fp8e4 DoubleRow MM [K=256 x N=512]: ~213-245 ns in-stream.
    Per tile: 8 (layer01) + 32 (layer2) + 4 (node-score M=1) = 44 MMs.
  - engine drains cost ~600-700 ns per [128,512] chunk AND steal PE time
    roughly proportional to bytes read -> drains read PSUM as bf16-hi
    (bitcast + stride-2, i.e. bf16 truncation; exact silu input err <0.4%).
  - weights prescaled x16 on host (fp8 subnormal avoidance); ACT silu
    applies scale=1/16.  b1/b2 are zero in this problem: no-bias 2-chunk
    silu ops; a `biased` fallback keeps general correctness.
  - ACT is the co-bottleneck (~16 chunk-silus/tile); DVE does the segment
    mean-pool reduces (fp8 SBUF reads) + ns staging; Pool/GpSimd does
    weight DMAs only; node scores run on the PE as M=1 DR MMs.
  - a segment (4 tiles) is issued as one block so layer01 MMs of later
    tiles cover the silu-drain latency of earlier ones.

Numerics (numpy model incl. x16 prescale + bf16 truncation + fp8 rounding):
max rel err 7.9e-4 vs f32 reference (gate 2e-2); HW-measured 7.96e-4.

Measured HW exec time 441-525 us/iter (device thermal state dependent;
R1-vs-R3000 repeat-loop deltas, median of 7) vs 1747 us for the f32r
baseline.  Floor analysis: PE 44 DR-MMs/tile ~9.4 us, ACT 16 silu
chunks/tile ~10.7 us; ACT is the binding engine at ~343 us total.
"""

import numpy as np

import concourse.bass as bass
import concourse.mybir as mybir
import concourse.tile as tile
from concourse import bacc
from concourse.bass import ts
from concourse.bass_utils import run_bass_kernel_spmd

B, L, H, D = 64, 2048, 1024, 128
NCORES = 8
B_LOC = B // NCORES          # segments per core
N_LOC = B_LOC * L            # tokens per core
T = 512                      # tokens per tile
NT = N_LOC // T              # tiles per core
U = L // T                   # tiles per segment
HC = H // 128                # hidden chunks
KP = HC // 2                 # k-pair chunks (256-wide DoubleRow contractions)
PRE = 16.0                   # host weight prescale (silu scale = 1/PRE)

F32 = mybir.dt.float32
BF16 = mybir.dt.bfloat16
FP8 = mybir.dt.float8e4
AF = mybir.ActivationFunctionType
AX = mybir.AxisListType
DR = mybir.MatmulPerfMode.DoubleRow

_NC_CACHE = {}


def _hi_bf16(pm):
    """View a [128, c, T] f32 PSUM tile as its bf16 high halves (truncation)."""
    return pm.bitcast(BF16).rearrange("p c (t two) -> p c t two",
                                      two=2)[:, :, :, 1]


def build_nc(reps=1, **vkw):
    key = (reps, tuple(sorted(vkw.items())))
    if key in _NC_CACHE:
        return _NC_CACHE[key]
    pmm_bufs = vkw.get("pmm_bufs", 3)   # [128, 2, T] f32: 2 banks each
    pns_bufs = vkw.get("pns_bufs", 2)
    raw_bufs = vkw.get("raw_bufs", 8)
    h1_bufs = vkw.get("h1_bufs", 5)
    h2_bufs = vkw.get("h2_bufs", 5)
    nsx_bufs = vkw.get("nsx_bufs", 3)
    biased = vkw.get("biased", False)
    no_pool = vkw.get("no_pool", False)
    no_ns = vkw.get("no_ns", False)
    no_heads = vkw.get("no_heads", False)
    defer = vkw.get("defer", True)
    if defer:
        h2_bufs = vkw.get("h2_bufs", 9)
    wide = vkw.get("wide", False)   # [128,4,T] PSUM tiles, silu4 drains
    if wide:
        pmm_bufs = vkw.get("pmm_bufs", 2)
        pns_bufs = 0

    nc = bacc.Bacc("TRN2", target_bir_lowering=False, debug=False,
                   num_devices=NCORES)

    x_d = nc.dram_tensor("xb", [128, NT, 2, T], FP8, kind="ExternalInput").ap()
    wd_d = nc.dram_tensor("wd", [128, 2, H], FP8, kind="ExternalInput").ap()
    w2_d = nc.dram_tensor("w2p", [128, KP, 2, H], FP8,
                          kind="ExternalInput").ap()
    b1_d = nc.dram_tensor("b1r", [128, HC], F32, kind="ExternalInput").ap()
    b2_d = nc.dram_tensor("b2r", [128, HC], F32, kind="ExternalInput").ap()
    wns8_d = nc.dram_tensor("wns8", [128, KP, 2, 16], FP8,
                            kind="ExternalInput").ap()
    wsp_d = nc.dram_tensor("wstr", [128, HC, 2], F32,
                           kind="ExternalInput").ap()
    out_d = nc.dram_tensor("out", [B_LOC, L + 1], F32,
                           kind="ExternalOutput").ap()

    with tile.TileContext(nc) as tc:
        with (
            tc.tile_pool(name="const", bufs=1) as cpool,
            tc.tile_pool(name="raw", bufs=raw_bufs) as raw,
            tc.tile_pool(name="h1", bufs=h1_bufs) as h1p,
            tc.tile_pool(name="h2", bufs=h2_bufs) as h2p,
            tc.tile_pool(name="acc", bufs=2) as accp,
            tc.tile_pool(name="nsx", bufs=nsx_bufs) as nsxp,
            tc.tile_pool(name="nsb", bufs=1) as nsp,
            tc.tile_pool(name="head", bufs=1) as headp,
            tc.tile_pool(name="pmm", bufs=pmm_bufs, space="PSUM") as pmm,
            tc.tile_pool(name="pns", bufs=max(pns_bufs, 1),
                         space="PSUM") as pnsp,
        ):
            if wide:
                pnsp = None
            wd = cpool.tile([128, 2, H], FP8)
            nc.gpsimd.dma_start(out=wd, in_=wd_d)
            w2 = cpool.tile([128, KP, 2, H], FP8)
            nc.gpsimd.dma_start(out=w2, in_=w2_d)
            b1 = cpool.tile([128, HC], F32)
            nc.gpsimd.dma_start(out=b1, in_=b1_d)
            b2 = cpool.tile([128, HC], F32)
            nc.gpsimd.dma_start(out=b2, in_=b2_d)
            wns8 = cpool.tile([128, KP, 2, 16], FP8)
            nc.gpsimd.dma_start(out=wns8, in_=wns8_d)
            wsp = cpool.tile([128, HC, 2], F32)
            nc.gpsimd.dma_start(out=wsp, in_=wsp_d)

            def load_x(t):
                xt = raw.tile([128, 2, T], FP8, tag="xt")
                nc.sync.dma_start(out=xt, in_=x_d[:, t, :, :])
                return xt

            def drain_silu(hf, g, pm, b, nch):
                # silu over nch chunks; PSUM read as truncated bf16
                if biased:
                    for c in range(nch):
                        m = nch * g + c
                        nc.scalar.activation(hf[:, m, :],
                                             _hi_bf16(pm)[:, c, :],
                                             AF.Silu, bias=b[:, m:m + 1],
                                             scale=1.0 / PRE)
                else:
                    nc.scalar.activation(hf[:, nch * g:nch * g + nch, :],
                                         _hi_bf16(pm), AF.Silu,
                                         scale=1.0 / PRE)

            def layer01(xt):
                # h1.T = silu((A1|B1) @ (xs|xn).T / PRE): one K=256 DR MM
                # per m-chunk, two chunks per 2-bank PSUM tile
                h1 = h1p.tile([128, KP, 2, T], FP8, tag="h1")
                h1f = h1.rearrange("p k two t -> p (k two) t")
                nch = 4 if wide else 2
                for g in range(HC // nch):
                    pm = pmm.tile([128, nch, T], F32, tag="mm")
                    for c in range(nch):
                        m = nch * g + c
                        nc.tensor.matmul(pm[:, c, :], wd[:, :, ts(m, 128)],
                                         xt, start=True, stop=True,
                                         perf_mode=DR)
                    drain_silu(h1f, g, pm, b1, nch)
                return h1

            def layer2(h1):
                h2 = h2p.tile([128, KP, 2, T], FP8, tag="h2")
                h2f = h2.rearrange("p k two t -> p (k two) t")
                nch = 4 if wide else 2
                for g in range(HC // nch):
                    pm = pmm.tile([128, nch, T], F32, tag="mm")
                    for c in range(nch):
                        m = nch * g + c
                        for k in range(KP):
                            nc.tensor.matmul(pm[:, c, :],
                                             w2[:, k, :, ts(m, 128)],
                                             h1[:, k, :, :],
                                             start=(k == 0),
                                             stop=(k == KP - 1),
                                             perf_mode=DR)
                    drain_silu(h2f, g, pm, b2, nch)
                return h2

            def pool_contrib(h2, stop_acc, u):
                # segment mean-pool: sum h2 over tokens (DVE, fp8 SBUF reads)
                h2f = h2.rearrange("p k two t -> p (k two) t")
                for m in range(HC):
                    nc.vector.reduce_sum(stop_acc[:, m, u:u + 1],
                                         h2f[:, m, :], axis=AX.X)

            def ns_chain(h2):
                # node scores on the PE: ns = w_ns . h2 via KP M=1 DR MMs
                if wide:
                    pnst = pmm.tile([128, 4, T], F32, tag="mm")
                    pns = pnst[0:1, 0, :]
                else:
                    pns = pnsp.tile([1, T], F32, tag="pns")
                for k in range(KP):
                    nc.tensor.matmul(pns, wns8[:, k, :, 0:1], h2[:, k, :, :],
                                     start=(k == 0), stop=(k == KP - 1),
                                     perf_mode=DR)
                return pns

            def ns_finish(pns, ns_all, s, u):
                ns_stage = nsxp.tile([1, T], F32, tag="ns_stage")
                nc.vector.tensor_copy(ns_stage, pns)
                nc.sync.dma_start(out=ns_all[s:s + 1, ts(u, T)], in_=ns_stage)

            def main_body():
                ns_all = nsp.tile([B_LOC, L], F32, tag="ns_all")
                stopT = nsp.tile([128, HC, B_LOC], F32, tag="stopT")

                def tail_work(h2s, stop_acc, s):
                    # consumers of segment s's h2 tiles: issued one segment
                    # late so the PE never waits on the silu drain tail
                    if no_pool:
                        nc.vector.memset(stop_acc, 0.0)
                    else:
                        for u in range(U):
                            pool_contrib(h2s[u], stop_acc, u)
                    if not no_ns:
                        pnss = [ns_chain(h2s[u]) for u in range(U)]
                        for u in range(U):
                            ns_finish(pnss[u], ns_all, s, u)
                    elif s == 0:
                        nc.vector.memset(ns_all, 0.0)
                    nc.vector.reduce_sum(stopT[:, :, s:s + 1], stop_acc,
                                         axis=AX.X)

                pend = [load_x(u) for u in range(U)]
                prev = None
                for s in range(B_LOC):
                    stop_acc = accp.tile([128, HC, U], F32, tag="stop_acc")
                    xts = pend
                    if s + 1 < B_LOC:
                        pend = [load_x((s + 1) * U + u) for u in range(U)]
                    h1s = [layer01(xts[u]) for u in range(U)]
                    if prev is not None:
                        tail_work(*prev)
                    h2s = [layer2(h1s[u]) for u in range(U)]
                    if defer:
                        prev = (h2s, stop_acc, s)
                    else:
                        tail_work(h2s, stop_acc, s)
                if prev is not None:
                    tail_work(*prev)

                if no_heads:
                    outsb0 = headp.tile([B_LOC, L + 1], F32, tag="outsb")
                    nc.vector.tensor_copy(outsb0[:, 0:L], ns_all)
                    nc.sync.dma_start(out=out_d, in_=outsb0)
                    return
                # ---- heads (batched over the 8 local segments) ----
                if wide:
                    pstt = pmm.tile([128, 4, T], F32, tag="mm")
                    pst = pstt[0:B_LOC, 0, 0:2]
                else:
                    pst = pnsp.tile([B_LOC, 2], F32, tag="pns")
                for k in range(HC):
                    nc.tensor.matmul(pst, stopT[:, k, :], wsp[:, k, :],
                                     start=(k == 0), stop=(k == HC - 1))
                st = headp.tile([B_LOC, 2], F32, tag="st")
                nc.scalar.mul(st, pst, 1.0 / L)
                negm = headp.tile([B_LOC, 1], F32, tag="negm")
                nc.vector.reduce_max(negm, st, axis=AX.X, negate=True)
                est = headp.tile([B_LOC, 2], F32, tag="est")
                sst = headp.tile([B_LOC, 1], F32, tag="sst")
                nc.scalar.activation(est, st, AF.Exp, bias=negm, scale=1.0,
                                     accum_out=sst)
                lst = headp.tile([B_LOC, 1], F32, tag="lst")
                nc.scalar.activation(lst, sst, AF.Ln)
                stop0 = headp.tile([B_LOC, 1], F32, tag="stop0")
                nc.vector.tensor_add(stop0, st[:, 0:1], negm)
                stop0b = headp.tile([B_LOC, 1], F32, tag="stop0b")
                nc.vector.tensor_sub(stop0b, stop0, lst)
                stop1 = headp.tile([B_LOC, 1], F32, tag="stop1")
                nc.vector.tensor_add(stop1, st[:, 1:2], negm)
                stop1b = headp.tile([B_LOC, 1], F32, tag="stop1b")
                nc.vector.tensor_sub(stop1b, stop1, lst)

                negnm = headp.tile([B_LOC, 1], F32, tag="negnm")
                nc.vector.reduce_max(negnm, ns_all, axis=AX.X, negate=True)
                esc = headp.tile([B_LOC, L], F32, tag="esc")
                nsum = headp.tile([B_LOC, 1], F32, tag="nsum")
                nc.scalar.activation(esc, ns_all, AF.Exp, bias=negnm,
                                     scale=1.0, accum_out=nsum)
                nls = headp.tile([B_LOC, 1], F32, tag="nls")
                nc.scalar.activation(nls, nsum, AF.Ln)
                fb = headp.tile([B_LOC, 1], F32, tag="fb")
                nc.vector.tensor_add(fb, stop0b, negnm)
                fb2 = headp.tile([B_LOC, 1], F32, tag="fb2")
                nc.vector.tensor_sub(fb2, fb, nls)

                outsb = headp.tile([B_LOC, L + 1], F32, tag="outsb")
                nc.scalar.activation(outsb[:, 0:L], ns_all, AF.Identity,
                                     bias=fb2, scale=1.0)
                nc.vector.tensor_copy(outsb[:, L:L + 1], stop1b)
                nc.sync.dma_start(out=out_d, in_=outsb)

            if reps == 1:
                main_body()
            else:
                with tc.For_i(0, reps, 1) as _i:
                    main_body()

    nc.compile()
    _NC_CACHE[key] = nc
    return nc


def _pad16(a):
    out = np.zeros(a.shape + (16,), dtype=a.dtype)
    out[..., 0] = a
    return out


def _prep_in_maps(x_seeds, x_nodes, W_seed, W_node, W1, b1, W2, b2, w_ns,
                  W_stop):
    import ml_dtypes
    E4 = ml_dtypes.float8_e4m3
    f32 = lambda a: np.asarray(a, dtype=np.float32)
    f32c = lambda a: np.ascontiguousarray(f32(a))

    # fold layers 0+1: A1 = W1 @ W_seed, B1 = W1 @ W_node  (x PRE prescale)
    A1 = (f32(W1) @ f32(W_seed)) * PRE        # [H, D]
    B1 = (f32(W1) @ f32(W_node)) * PRE
    wd = np.stack([A1.T, B1.T], axis=1)       # [D, 2, H]
    w2p = ((f32(W2) * PRE).T.reshape(KP, 2, 128, H).transpose(2, 0, 1, 3))
    shared = {
        "wd": np.ascontiguousarray(wd.astype(E4)),
        "w2p": np.ascontiguousarray(w2p.astype(E4)),
        "b1r": f32c(f32(b1).reshape(HC, 128).T),
        "b2r": f32c(f32(b2).reshape(HC, 128).T),
        "wns8": _pad16(f32(w_ns).reshape(KP, 2, 128).transpose(2, 0, 1)
                       .astype(E4)),
        "wstr": f32c(f32(W_stop).reshape(2, HC, 128).transpose(2, 1, 0)),
    }
    xs_q = f32(x_seeds).astype(E4)
    xn_q = f32(x_nodes).astype(E4)
    in_maps = []
    for cidx in range(NCORES):
        sl = slice(cidx * N_LOC, (cidx + 1) * N_LOC)
        a = xs_q[sl].reshape(NT, T, D).transpose(2, 0, 1)   # [128, NT, T]
        b = xn_q[sl].reshape(NT, T, D).transpose(2, 0, 1)
        xb = np.ascontiguousarray(np.stack([a, b], axis=2))  # [128, NT, 2, T]
        m = {"xb": xb}
        m.update(shared)
        in_maps.append(m)
    biased = bool(np.any(np.asarray(b1)) or np.any(np.asarray(b2)))
    return in_maps, biased


def run_on_hw(in_maps, reps=1, biased=False):
    nc = build_nc(reps, **({"biased": True} if biased else {}))
    res = run_bass_kernel_spmd(nc, in_maps, core_ids=list(range(NCORES)))
    return res


def kernel(x_seeds, x_nodes, W_seed, W_node, W1, b1, W2, b2, w_ns, W_stop,
           indptr=None, **_unused):
    in_maps, biased = _prep_in_maps(x_seeds, x_nodes, W_seed, W_node, W1, b1,
                                    W2, b2, w_ns, W_stop)
    res = run_on_hw(in_maps, biased=biased)
    out = np.concatenate([res.results[c]["out"] for c in range(NCORES)],
                         axis=0)
    return out.astype(np.float32)
